# revision 1
# baseline (speedup 1.0000x reference)
"""Multi-head attention (B=16, N=1024, E=768, H=8) on 8 Trainium2 NeuronCores.

Strategy: data-parallel over batch (2 batches per core, no collectives).
Per core, a fused attention kernel:
  - host pre-transposes x -> x^T and pre-permutes the interleaved qkv weights
  - Q^T/K^T per (batch, head) via matmul (contraction over E on partitions)
  - V computed per batch for all heads, padded to 128 cols per head with a
    ones-column at col 96 (so the attention row-sum falls out of the O matmul)
  - S^T = (K^T)^T @ Q^T  -> PSUM, Exp on scalar engine -> SBUF (fp32r)
  - O'' = V''^T @ P^T accumulated over key chunks; row 96 = softmax denom
  - normalize via reciprocal + gpsimd partition_broadcast + fused DVE op
  - final projection from the transposed O layout; bias added on DVE
All matmuls run in fp32r (full PE rate at free-dim >= 256).
"""
import sys
import os

for _p in ("/opt/trn_rl_repo", "/root/.axon_site", "/root/.axon_site/_ro/trn_rl_repo"):
    if os.path.isdir(_p) and _p not in sys.path:
        sys.path.append(_p)

import numpy as np

B, N, E, H = 16, 1024, 768, 8
D = E // H            # 96
NCORES = 8
BPC = B // NCORES     # batches per core = 2
EC = E // 128         # 6 E-chunks
TC = N // 128         # 8 token chunks
DP = 128              # padded per-head width in V/proj layouts
SCALE = float(1.0 / np.sqrt(np.float32(E)))

_NC_CACHE = {}


def _build_nc():
    import concourse.bacc as bacc
    import concourse.mybir as mybir
    import concourse.tile as tile

    FP32 = mybir.dt.float32
    FP32R = mybir.dt.float32r
    AF = mybir.ActivationFunctionType
    OP = mybir.AluOpType

    nc = bacc.Bacc("TRN2", target_bir_lowering=False, debug=False, num_devices=NCORES)

    xt = nc.dram_tensor("xt", [BPC, 128, EC * N], FP32R, kind="ExternalInput")
    wq = nc.dram_tensor("wq", [H, 128, EC * D], FP32R, kind="ExternalInput")
    wk = nc.dram_tensor("wk", [H, 128, EC * D], FP32R, kind="ExternalInput")
    wv = nc.dram_tensor("wv", [128, EC * H * DP], FP32R, kind="ExternalInput")
    vb = nc.dram_tensor("vb", [128, H * DP], FP32, kind="ExternalInput")
    bqk = nc.dram_tensor("bqk", [128, 2 * H], FP32, kind="ExternalInput")
    pw = nc.dram_tensor("pw", [128, H * E], FP32R, kind="ExternalInput")
    pb = nc.dram_tensor("pb", [128, E], FP32, kind="ExternalInput")
    out = nc.dram_tensor("out", [BPC, N, E], FP32, kind="ExternalOutput")

    with tile.TileContext(nc) as tc:
        with (
            tc.tile_pool(name="const", bufs=1) as const,
            tc.tile_pool(name="xtp", bufs=1) as xtp,
            tc.tile_pool(name="vp", bufs=1) as vp,
            tc.tile_pool(name="oallp", bufs=1) as oallp,
            tc.tile_pool(name="wqp", bufs=2) as wqp,
            tc.tile_pool(name="wkp", bufs=2) as wkp,
            tc.tile_pool(name="qtp", bufs=2) as qtp,
            tc.tile_pool(name="ktp", bufs=2) as ktp,
            tc.tile_pool(name="estp", bufs=3) as estp,
            tc.tile_pool(name="rp", bufs=2) as rp,
            tc.tile_pool(name="rbcp", bufs=2) as rbcp,
            tc.tile_pool(name="obp", bufs=2) as obp,
            tc.tile_pool(name="vgps", bufs=1, space="PSUM") as vgps,
            tc.tile_pool(name="qkps", bufs=1, space="PSUM") as qkps,
            tc.tile_pool(name="stps", bufs=2, space="PSUM") as stps,
            tc.tile_pool(name="ops", bufs=2, space="PSUM") as ops,
        ):
            # ---- resident constants ----
            wv_sb = const.tile([128, EC * H * DP], FP32R)
            for c in range(EC):
                nc.sync.dma_start(
                    wv_sb[:, c * H * DP:(c + 1) * H * DP],
                    wv.ap()[:, c * H * DP:(c + 1) * H * DP],
                )
            pw_sb = const.tile([128, H * E], FP32R)
            for hc in range(H):
                nc.sync.dma_start(
                    pw_sb[:, hc * E:(hc + 1) * E], pw.ap()[:, hc * E:(hc + 1) * E]
                )
            vb_sb = const.tile([128, H * DP], FP32)
            nc.sync.dma_start(vb_sb[:], vb.ap())
            pb_sb = const.tile([128, E], FP32)
            nc.sync.dma_start(pb_sb[:], pb.ap())
            bqk_sb = const.tile([128, 2 * H], FP32)
            nc.sync.dma_start(bqk_sb[:], bqk.ap())

            for b in range(BPC):
                # ---- x^T for this batch ----
                xt_sb = xtp.tile([128, EC * N], FP32R, tag="xt")
                for c in range(EC):
                    nc.sync.dma_start(
                        xt_sb[:, c * N:(c + 1) * N], xt.ap()[b, :, c * N:(c + 1) * N]
                    )

                # ---- V'' for all heads: [tok128 x (tokchunk, head, 128)] ----
                v_sb = vp.tile([128, TC * H * DP], FP32R, tag="v")
                for t in range(TC):
                    vg = vgps.tile([128, H * DP], FP32, tag="vg")
                    for c in range(EC):
                        for nh in range(2):
                            nc.tensor.matmul(
                                vg[:, nh * 512:(nh + 1) * 512],
                                xt_sb[:, c * N + t * 128: c * N + (t + 1) * 128],
                                wv_sb[:, c * H * DP + nh * 512: c * H * DP + (nh + 1) * 512],
                                start=(c == 0),
                                stop=(c == EC - 1),
                            )
                    nc.vector.tensor_tensor(
                        v_sb[:, t * H * DP:(t + 1) * H * DP], vg[:], vb_sb[:],
                        op=OP.add,
                    )

                # ---- O_all^T accumulator in padded-head layout ----
                o_all = oallp.tile([128, H * N], FP32R, tag="oall")

                for h in range(H):
                    wqh = wqp.tile([128, EC * D], FP32R, tag="wq")
                    nc.sync.dma_start(wqh[:], wq.ap()[h])
                    wkh = wkp.tile([128, EC * D], FP32R, tag="wk")
                    nc.sync.dma_start(wkh[:], wk.ap()[h])

                    # Q^T then K^T (shared psum slot)
                    q_ps = qkps.tile([D, N], FP32, tag="qk")
                    for c in range(EC):
                        for qh in range(2):
                            nc.tensor.matmul(
                                q_ps[:, qh * 512:(qh + 1) * 512],
                                wqh[:, c * D:(c + 1) * D],
                                xt_sb[:, c * N + qh * 512: c * N + (qh + 1) * 512],
                                start=(c == 0),
                                stop=(c == EC - 1),
                            )
                    qt = qtp.tile([D, N], FP32R, tag="qt")
                    nc.scalar.add(qt[:], q_ps[:], bqk_sb[0:D, 2 * h:2 * h + 1])

                    k_ps = qkps.tile([D, N], FP32, tag="qk")
                    for c in range(EC):
                        for qh in range(2):
                            nc.tensor.matmul(
                                k_ps[:, qh * 512:(qh + 1) * 512],
                                wkh[:, c * D:(c + 1) * D],
                                xt_sb[:, c * N + qh * 512: c * N + (qh + 1) * 512],
                                start=(c == 0),
                                stop=(c == EC - 1),
                            )
                    kt = ktp.tile([D, N], FP32R, tag="kt")
                    nc.scalar.add(kt[:], k_ps[:], bqk_sb[0:D, 2 * h + 1:2 * h + 2])

                    for qh in range(2):
                        # S^T per key-chunk, exp, then O accumulation
                        ests = []
                        for t in range(TC):
                            st = stps.tile([128, 512], FP32, tag="st")
                            nc.tensor.matmul(
                                st[:],
                                kt[:, t * 128:(t + 1) * 128],
                                qt[:, qh * 512:(qh + 1) * 512],
                                start=True,
                                stop=True,
                            )
                            est = estp.tile([128, 512], FP32R, tag="est")
                            nc.scalar.activation(est[:], st[:], AF.Exp)
                            ests.append(est)

                        o_ps = ops.tile([128, 512], FP32, tag="o")
                        for t in range(TC):
                            nc.tensor.matmul(
                                o_ps[:],
                                v_sb[:, t * H * DP + h * DP: t * H * DP + (h + 1) * DP],
                                ests[t][:],
                                start=(t == 0),
                                stop=(t == TC - 1),
                            )

                        r = rp.tile([1, 512], FP32, tag="r")
                        nc.vector.reciprocal(r[:], o_ps[D:D + 1, :])
                        rbc = rbcp.tile([128, 512], FP32, tag="rbc")
                        nc.gpsimd.partition_broadcast(rbc[:], r[:])
                        nc.vector.scalar_tensor_tensor(
                            o_all[:, h * N + qh * 512: h * N + (qh + 1) * 512],
                            o_ps[:],
                            SCALE,
                            rbc[:],
                            OP.mult,
                            OP.mult,
                        )

                # ---- output projection ----
                for t in range(TC):
                    pj = vgps.tile([128, H * DP], FP32, tag="vg")
                    for hc in range(H):
                        lhsT = o_all[:, hc * N + t * 128: hc * N + (t + 1) * 128]
                        nc.tensor.matmul(
                            pj[:, 0:512], lhsT, pw_sb[:, hc * E: hc * E + 512],
                            start=(hc == 0), stop=(hc == H - 1),
                        )
                        nc.tensor.matmul(
                            pj[:, 512:E], lhsT, pw_sb[:, hc * E + 512: hc * E + E],
                            start=(hc == 0), stop=(hc == H - 1),
                        )
                    ob = obp.tile([128, E], FP32, tag="ob")
                    nc.vector.tensor_tensor(ob[:], pj[:, 0:E], pb_sb[:], op=OP.add)
                    nc.sync.dma_start(out.ap()[b, t * 128:(t + 1) * 128, :], ob[:])

    nc.compile()
    return nc


def get_nc():
    if "nc" not in _NC_CACHE:
        _NC_CACHE["nc"] = _build_nc()
    return _NC_CACHE["nc"]


def _prep_inputs(x, qkv_w, qkv_b, proj_w, proj_b):
    """Host-side layout prep shared by all cores + per-core x shards."""
    x = np.ascontiguousarray(x, dtype=np.float32)
    qkv_w = np.asarray(qkv_w, dtype=np.float32)
    qkv_b = np.asarray(qkv_b, dtype=np.float32)
    proj_w = np.asarray(proj_w, dtype=np.float32)
    proj_b = np.asarray(proj_b, dtype=np.float32)

    hh = np.arange(H)[:, None]
    dd = np.arange(D)[None, :]
    idx = [(hh * 3 * D + dd * 3 + c).reshape(-1) for c in range(3)]  # [768] each

    # wq/wk: [H, 128, EC*D]; wX_l[h, p, c*D+d] = qkv_w[idx[c_][h*D+d], c*128+p]
    def qk_layout(cix):
        wT = qkv_w[idx[cix], :].T  # [E, 768] : [e, h*D+d]
        w = wT.reshape(EC, 128, H, D)          # [c, p, h, d]
        return np.ascontiguousarray(w.transpose(2, 1, 0, 3).reshape(H, 128, EC * D))

    wq_l = qk_layout(0)
    wk_l = qk_layout(1)

    # wv: [128, EC*H*DP]; col c*H*DP + h*DP + d = qkv_w[idx2[h*D+d], c*128+p], pad 0
    wvT = qkv_w[idx[2], :].T.reshape(EC, 128, H, D)  # [c, p, h, d]
    wv_l = np.zeros((128, EC, H, DP), dtype=np.float32)
    wv_l[:, :, :, :D] = wvT.transpose(1, 0, 2, 3)
    wv_l = np.ascontiguousarray(wv_l.reshape(128, EC * H * DP))

    # vb: [128, H*DP] broadcast v-bias + ones column at d=D
    vb_row = np.zeros((H, DP), dtype=np.float32)
    vb_row[:, :D] = qkv_b[idx[2]].reshape(H, D)
    vb_row[:, D] = 1.0
    vb_l = np.ascontiguousarray(np.broadcast_to(vb_row.reshape(1, H * DP), (128, H * DP)))

    # bqk: [128, 2H]; col 2h = q bias, col 2h+1 = k bias (rows 0..D-1)
    bqk_l = np.zeros((128, 2 * H), dtype=np.float32)
    bqk_l[:D, 0::2] = qkv_b[idx[0]].reshape(H, D).T
    bqk_l[:D, 1::2] = qkv_b[idx[1]].reshape(H, D).T

    # pw: [128, H*E]; pw_l[p, h*E+e] = proj_w[e, h*D+dd] for p=dd<D else 0
    pw_l = np.zeros((128, H, E), dtype=np.float32)
    pw_l[:D, :, :] = proj_w.reshape(E, H, D).transpose(2, 1, 0)
    pw_l = np.ascontiguousarray(pw_l.reshape(128, H * E))

    pb_l = np.ascontiguousarray(np.broadcast_to(proj_b.reshape(1, E), (128, E)))

    # x^T per batch in sbuf layout: [B, 128, EC*N]; [b, p, c*N+n] = x[b, n, c*128+p]
    xt_all = np.ascontiguousarray(
        x.reshape(B, N, EC, 128).transpose(0, 3, 2, 1).reshape(B, 128, EC * N)
    )

    in_maps = []
    for core in range(NCORES):
        xt_core = np.ascontiguousarray(
            xt_all[core * BPC:(core + 1) * BPC]
        )
        in_maps.append(
            {
                "xt": xt_core,
                "wq": wq_l,
                "wk": wk_l,
                "wv": wv_l,
                "vb": vb_l,
                "bqk": bqk_l,
                "pw": pw_l,
                "pb": pb_l,
            }
        )
    return in_maps


def run(inputs, trace=False):
    from concourse.bass_utils import run_bass_kernel_spmd

    nc = get_nc()
    in_maps = _prep_inputs(**inputs)
    res = run_bass_kernel_spmd(
        nc, in_maps, core_ids=list(range(NCORES)), trace=trace
    )
    out = np.concatenate([res.results[c]["out"] for c in range(NCORES)], axis=0)
    return out, res


def kernel(**inputs) -> np.ndarray:
    out, _ = run(inputs, trace=False)
    return out


# revision 5
# speedup vs baseline: 1.2738x; 1.2738x over previous
"""Multi-head attention (B=16, N=1024, E=768, H=8) on 8 Trainium2 NeuronCores.

Strategy: data-parallel over batch (2 batches per core, no collectives).
Per core, a fused attention kernel:
  - host pre-transposes x -> x^T and pre-permutes the interleaved qkv weights
  - Q^T/K^T per (batch, head) via matmul (contraction over E on partitions),
    computed in 512-column halves so PSUM->SBUF bias-copies pipeline
  - V computed per batch for all heads in bf16, padded to 128 cols per head
    with a ones-column at col 96 (attention row-sums fall out of the O matmul)
  - S^T = (K^T)^T @ Q^T -> PSUM (fp32r matmul), Exp on scalar engine -> bf16
  - O'' = V''^T @ P^T (bf16 matmuls) accumulated over key chunks, software-
    pipelined with the S matmuls; row 96 = softmax denominator
  - normalize via reciprocal_approx_fast + gpsimd partition_broadcast + one
    fused DVE multiply (folds the 1/sqrt(E) post-softmax scale)
  - final projection from the transposed O layout; bias added on DVE
"""
import sys
import os

for _p in ("/opt/trn_rl_repo", "/root/.axon_site", "/root/.axon_site/_ro/trn_rl_repo"):
    if os.path.isdir(_p) and _p not in sys.path:
        sys.path.append(_p)

import numpy as np

B, N, E, H = 16, 1024, 768, 8
D = E // H            # 96
NCORES = 8
BPC = B // NCORES     # batches per core = 2
EC = E // 128         # 6 E-chunks
TC = N // 128         # 8 token chunks
DP = 128              # padded per-head width in V/proj layouts
SCALE = float(1.0 / np.sqrt(np.float32(E)))

_NC_CACHE = {}


def _build_nc():
    import concourse.bacc as bacc
    import concourse.mybir as mybir
    import concourse.tile as tile

    FP32 = mybir.dt.float32
    FP32R = mybir.dt.float32r
    BF16 = mybir.dt.bfloat16
    AF = mybir.ActivationFunctionType
    OP = mybir.AluOpType

    nc = bacc.Bacc("TRN2", target_bir_lowering=False, debug=False, num_devices=NCORES)

    xt = nc.dram_tensor("xt", [BPC, 128, EC * N], FP32R, kind="ExternalInput")
    wq = nc.dram_tensor("wq", [H, 128, EC * D], FP32R, kind="ExternalInput")
    wk = nc.dram_tensor("wk", [H, 128, EC * D], FP32R, kind="ExternalInput")
    wv = nc.dram_tensor("wv", [128, EC * H * DP], FP32R, kind="ExternalInput")
    vb = nc.dram_tensor("vb", [128, H * DP], FP32, kind="ExternalInput")
    bqk = nc.dram_tensor("bqk", [128, 2 * H], FP32, kind="ExternalInput")
    pw = nc.dram_tensor("pw", [128, H * E], FP32R, kind="ExternalInput")
    pb = nc.dram_tensor("pb", [128, E], FP32, kind="ExternalInput")
    out = nc.dram_tensor("out", [BPC, N, E], FP32, kind="ExternalOutput")

    from contextlib import ExitStack

    with tile.TileContext(nc) as tc:
        with ExitStack() as ctx:
            const = ctx.enter_context(tc.tile_pool(name="const", bufs=1))
            xtp = ctx.enter_context(tc.tile_pool(name="xtp", bufs=1))
            vp = ctx.enter_context(tc.tile_pool(name="vp", bufs=1))
            oallp = ctx.enter_context(tc.tile_pool(name="oallp", bufs=1))
            wqp = ctx.enter_context(tc.tile_pool(name="wqp", bufs=2))
            wkp = ctx.enter_context(tc.tile_pool(name="wkp", bufs=2))
            qtp = ctx.enter_context(tc.tile_pool(name="qtp", bufs=2))
            ktp = ctx.enter_context(tc.tile_pool(name="ktp", bufs=2))
            estp = ctx.enter_context(tc.tile_pool(name="estp", bufs=6))
            rp = ctx.enter_context(tc.tile_pool(name="rp", bufs=4))
            rbcp = ctx.enter_context(tc.tile_pool(name="rbcp", bufs=2))
            obp = ctx.enter_context(tc.tile_pool(name="obp", bufs=2))
            vgps = ctx.enter_context(tc.tile_pool(name="vgps", bufs=1, space="PSUM"))
            qkps = ctx.enter_context(tc.tile_pool(name="qkps", bufs=2, space="PSUM"))
            stps = ctx.enter_context(tc.tile_pool(name="stps", bufs=2, space="PSUM"))
            ops = ctx.enter_context(tc.tile_pool(name="ops", bufs=2, space="PSUM"))
            # ---- resident constants ----
            wv_sb = const.tile([128, EC * H * DP], FP32R)
            for c in range(EC):
                nc.sync.dma_start(
                    wv_sb[:, c * H * DP:(c + 1) * H * DP],
                    wv.ap()[:, c * H * DP:(c + 1) * H * DP],
                )
            pw_sb = const.tile([128, H * E], FP32R)
            for hc in range(H):
                nc.sync.dma_start(
                    pw_sb[:, hc * E:(hc + 1) * E], pw.ap()[:, hc * E:(hc + 1) * E]
                )
            vb_sb = const.tile([128, H * DP], FP32)
            nc.sync.dma_start(vb_sb[:], vb.ap())
            pb_sb = const.tile([128, E], FP32)
            nc.sync.dma_start(pb_sb[:], pb.ap())
            bqk_sb = const.tile([128, 2 * H], FP32)
            nc.sync.dma_start(bqk_sb[:], bqk.ap())

            for b in range(BPC):
                # ---- x^T for this batch ----
                xt_sb = xtp.tile([128, EC * N], FP32R, tag="xt")
                for c in range(EC):
                    nc.sync.dma_start(
                        xt_sb[:, c * N:(c + 1) * N], xt.ap()[b, :, c * N:(c + 1) * N]
                    )

                # ---- V'' for all heads (bf16): [tok128 x (tokchunk, head, 128)] ----
                v_sb = vp.tile([128, TC * H * DP], BF16, tag="v")
                for t in range(TC):
                    vg = vgps.tile([128, H * DP], FP32, tag="vg")
                    for c in range(EC):
                        for nh in range(2):
                            nc.tensor.matmul(
                                vg[:, nh * 512:(nh + 1) * 512],
                                xt_sb[:, c * N + t * 128: c * N + (t + 1) * 128],
                                wv_sb[:, c * H * DP + nh * 512: c * H * DP + (nh + 1) * 512],
                                start=(c == 0),
                                stop=(c == EC - 1),
                            )
                    nc.vector.tensor_tensor(
                        v_sb[:, t * H * DP:(t + 1) * H * DP], vg[:], vb_sb[:],
                        op=OP.add,
                    )

                # ---- O_all^T accumulator in padded-head layout ----
                o_all = oallp.tile([128, H * N], FP32R, tag="oall")

                for h in range(H):
                    wqh = wqp.tile([128, EC * D], FP32R, tag="wq")
                    nc.sync.dma_start(wqh[:], wq.ap()[h])
                    wkh = wkp.tile([128, EC * D], FP32R, tag="wk")
                    nc.sync.dma_start(wkh[:], wk.ap()[h])

                    # Q^T then K^T in 512-col halves (pipelined psum->sbuf copies)
                    qt = qtp.tile([D, N], FP32R, tag="qt")
                    kt = ktp.tile([D, N], FP32R, tag="kt")
                    for dst, w_sb, bcol in ((qt, wqh, 2 * h), (kt, wkh, 2 * h + 1)):
                        for qh in range(2):
                            g_ps = qkps.tile([D, 512], FP32, tag="qk")
                            for c in range(EC):
                                nc.tensor.matmul(
                                    g_ps[:],
                                    w_sb[:, c * D:(c + 1) * D],
                                    xt_sb[:, c * N + qh * 512: c * N + (qh + 1) * 512],
                                    start=(c == 0),
                                    stop=(c == EC - 1),
                                )
                            nc.scalar.add(
                                dst[:, qh * 512:(qh + 1) * 512], g_ps[:],
                                bqk_sb[0:D, bcol:bcol + 1],
                            )

                    # S^T -> exp -> O, software pipelined over key chunks.
                    # Both q-halves share each weight load (same lhsT).
                    o_ps = [ops.tile([128, 512], FP32, tag="o", name=f"o_{b}_{h}_{i}") for i in range(2)]
                    ests = [[None] * TC, [None] * TC]

                    def s_step(t):
                        for qh in range(2):
                            st = stps.tile([128, 512], FP32, tag="st")
                            nc.tensor.matmul(
                                st[:],
                                kt[:, t * 128:(t + 1) * 128],
                                qt[:, qh * 512:(qh + 1) * 512],
                                start=True,
                                stop=True,
                            )
                            est = estp.tile([128, 512], BF16, tag="est")
                            nc.scalar.activation(est[:], st[:], AF.Exp)
                            ests[qh][t] = est

                    def o_step(t):
                        for qh in range(2):
                            nc.tensor.matmul(
                                o_ps[qh][:],
                                v_sb[:, t * H * DP + h * DP: t * H * DP + (h + 1) * DP],
                                ests[qh][t][:],
                                start=(t == 0),
                                stop=(t == TC - 1),
                            )

                    LAT = 1
                    for t in range(TC):
                        s_step(t)
                        if t >= LAT:
                            o_step(t - LAT)
                    for t in range(TC - LAT, TC):
                        o_step(t)

                    for qh in range(2):
                        r = rp.tile([1, 512], FP32, tag="r")
                        rs = rp.tile([1, 512], FP32, tag="rs")
                        nc.vector.reciprocal_approx_accurate(
                            r[:], o_ps[qh][0:1, :], rs[:]
                        )
                        rbc = rbcp.tile([128, 512], FP32, tag="rbc")
                        nc.gpsimd.partition_broadcast(rbc[:], r[:])
                        nc.vector.scalar_tensor_tensor(
                            o_all[:, h * N + qh * 512: h * N + (qh + 1) * 512],
                            o_ps[qh][:],
                            SCALE,
                            rbc[:],
                            OP.mult,
                            OP.mult,
                        )

                # ---- output projection ----
                for t in range(TC):
                    pj = vgps.tile([128, H * DP], FP32, tag="vg")
                    for hc in range(H):
                        lhsT = o_all[:, hc * N + t * 128: hc * N + (t + 1) * 128]
                        nc.tensor.matmul(
                            pj[:, 0:512], lhsT, pw_sb[:, hc * E: hc * E + 512],
                            start=(hc == 0), stop=(hc == H - 1),
                        )
                        nc.tensor.matmul(
                            pj[:, 512:E], lhsT, pw_sb[:, hc * E + 512: hc * E + E],
                            start=(hc == 0), stop=(hc == H - 1),
                        )
                    ob = obp.tile([128, E], FP32, tag="ob")
                    nc.vector.tensor_tensor(ob[:], pj[:, 0:E], pb_sb[:], op=OP.add)
                    nc.sync.dma_start(out.ap()[b, t * 128:(t + 1) * 128, :], ob[:])

    nc.compile()
    return nc


def get_nc():
    if "nc" not in _NC_CACHE:
        _NC_CACHE["nc"] = _build_nc()
    return _NC_CACHE["nc"]


def _prep_inputs(x, qkv_w, qkv_b, proj_w, proj_b):
    """Host-side layout prep shared by all cores + per-core x shards."""
    x = np.ascontiguousarray(x, dtype=np.float32)
    qkv_w = np.asarray(qkv_w, dtype=np.float32)
    qkv_b = np.asarray(qkv_b, dtype=np.float32)
    proj_w = np.asarray(proj_w, dtype=np.float32)
    proj_b = np.asarray(proj_b, dtype=np.float32)

    hh = np.arange(H)[:, None]
    dd = np.arange(D)[None, :]
    idx = [(hh * 3 * D + dd * 3 + c).reshape(-1) for c in range(3)]  # [768] each

    # wq/wk: [H, 128, EC*D]; wX_l[h, p, c*D+d] = qkv_w[idx[c_][h*D+d], c*128+p]
    def qk_layout(cix):
        wT = qkv_w[idx[cix], :].T  # [E, 768] : [e, h*D+d]
        w = wT.reshape(EC, 128, H, D)          # [c, p, h, d]
        return np.ascontiguousarray(w.transpose(2, 1, 0, 3).reshape(H, 128, EC * D))

    wq_l = qk_layout(0)
    wk_l = qk_layout(1)

    # wv: [128, EC*H*DP]; col c*H*DP + h*DP + d = qkv_w[idx2[h*D+d], c*128+p], pad 0
    wvT = qkv_w[idx[2], :].T.reshape(EC, 128, H, D)  # [c, p, h, d]
    wv_l = np.zeros((128, EC, H, DP), dtype=np.float32)
    wv_l[:, :, :, 1:D + 1] = wvT.transpose(1, 0, 2, 3)
    wv_l = np.ascontiguousarray(wv_l.reshape(128, EC * H * DP))

    # vb: [128, H*DP] broadcast v-bias + ones column at d=D
    vb_row = np.zeros((H, DP), dtype=np.float32)
    vb_row[:, 1:D + 1] = qkv_b[idx[2]].reshape(H, D)
    vb_row[:, 0] = 1.0
    vb_l = np.ascontiguousarray(np.broadcast_to(vb_row.reshape(1, H * DP), (128, H * DP)))

    # bqk: [128, 2H]; col 2h = q bias, col 2h+1 = k bias (rows 0..D-1)
    bqk_l = np.zeros((128, 2 * H), dtype=np.float32)
    bqk_l[:D, 0::2] = qkv_b[idx[0]].reshape(H, D).T
    bqk_l[:D, 1::2] = qkv_b[idx[1]].reshape(H, D).T

    # pw: [128, H*E]; pw_l[p, h*E+e] = proj_w[e, h*D+dd] for p=dd<D else 0
    pw_l = np.zeros((128, H, E), dtype=np.float32)
    pw_l[1:D + 1, :, :] = proj_w.reshape(E, H, D).transpose(2, 1, 0)
    pw_l = np.ascontiguousarray(pw_l.reshape(128, H * E))

    pb_l = np.ascontiguousarray(np.broadcast_to(proj_b.reshape(1, E), (128, E)))

    # x^T per batch in sbuf layout: [B, 128, EC*N]; [b, p, c*N+n] = x[b, n, c*128+p]
    xt_all = np.ascontiguousarray(
        x.reshape(B, N, EC, 128).transpose(0, 3, 2, 1).reshape(B, 128, EC * N)
    )

    in_maps = []
    for core in range(NCORES):
        xt_core = np.ascontiguousarray(
            xt_all[core * BPC:(core + 1) * BPC]
        )
        in_maps.append(
            {
                "xt": xt_core,
                "wq": wq_l,
                "wk": wk_l,
                "wv": wv_l,
                "vb": vb_l,
                "bqk": bqk_l,
                "pw": pw_l,
                "pb": pb_l,
            }
        )
    return in_maps


def run(inputs, trace=False):
    from concourse.bass_utils import run_bass_kernel_spmd

    nc = get_nc()
    in_maps = _prep_inputs(**inputs)
    res = run_bass_kernel_spmd(
        nc, in_maps, core_ids=list(range(NCORES)), trace=trace
    )
    out = np.concatenate([res.results[c]["out"] for c in range(NCORES)], axis=0)
    return out, res


def kernel(**inputs) -> np.ndarray:
    out, _ = run(inputs, trace=False)
    return out


# revision 7
# speedup vs baseline: 1.4027x; 1.1012x over previous
"""Multi-head attention (B=16, N=1024, E=768, H=8) on 8 Trainium2 NeuronCores.

Strategy: data-parallel over batch (2 batches per core, no collectives).
Per core, a fused attention kernel:
  - host pre-transposes x -> x^T and pre-permutes the interleaved qkv weights
  - Q^T/K^T per (batch, head) via matmul (contraction over E on partitions),
    computed in 512-column halves so PSUM->SBUF bias-copies pipeline
  - V computed per batch for all heads in bf16, padded to 128 cols per head
    with a ones-column at col 96 (attention row-sums fall out of the O matmul)
  - S^T = (K^T)^T @ Q^T -> PSUM (fp32r matmul), Exp on scalar engine -> bf16
  - O'' = V''^T @ P^T (bf16 matmuls) accumulated over key chunks, software-
    pipelined with the S matmuls; row 96 = softmax denominator
  - normalize via reciprocal_approx_fast + gpsimd partition_broadcast + one
    fused DVE multiply (folds the 1/sqrt(E) post-softmax scale)
  - final projection from the transposed O layout; bias added on DVE
"""
import sys
import os

for _p in ("/opt/trn_rl_repo", "/root/.axon_site", "/root/.axon_site/_ro/trn_rl_repo"):
    if os.path.isdir(_p) and _p not in sys.path:
        sys.path.append(_p)

import numpy as np

B, N, E, H = 16, 1024, 768, 8
D = E // H            # 96
NCORES = 8
BPC = B // NCORES     # batches per core = 2
EC = E // 128         # 6 E-chunks
TC = N // 128         # 8 token chunks
DP = 128              # padded per-head width in the proj layout
VW = D + 1            # per-head width in the V layout (ones col + 96 V cols)
SCALE = float(1.0 / np.sqrt(np.float32(E)))

_NC_CACHE = {}


def _build_nc():
    import concourse.bacc as bacc
    import concourse.mybir as mybir
    import concourse.tile as tile

    FP32 = mybir.dt.float32
    FP32R = mybir.dt.float32r
    BF16 = mybir.dt.bfloat16
    AF = mybir.ActivationFunctionType
    OP = mybir.AluOpType

    nc = bacc.Bacc("TRN2", target_bir_lowering=False, debug=False, num_devices=NCORES)

    xt = nc.dram_tensor("xt", [BPC, 128, EC * N], FP32R, kind="ExternalInput")
    wq = nc.dram_tensor("wq", [H, 128, EC * D], FP32R, kind="ExternalInput")
    wk = nc.dram_tensor("wk", [H, 128, EC * D], FP32R, kind="ExternalInput")
    wv = nc.dram_tensor("wv", [128, EC * H * VW], FP32R, kind="ExternalInput")
    vb = nc.dram_tensor("vb", [128, H * VW], FP32, kind="ExternalInput")
    bqk = nc.dram_tensor("bqk", [128, 2 * H], FP32, kind="ExternalInput")
    pw = nc.dram_tensor("pw", [128, H * E], FP32R, kind="ExternalInput")
    pb = nc.dram_tensor("pb", [128, E], FP32, kind="ExternalInput")
    out = nc.dram_tensor("out", [BPC, N, E], FP32, kind="ExternalOutput")

    from contextlib import ExitStack

    with tile.TileContext(nc) as tc:
        with ExitStack() as ctx:
            const = ctx.enter_context(tc.tile_pool(name="const", bufs=1))
            xtp = ctx.enter_context(tc.tile_pool(name="xtp", bufs=1))
            vp = ctx.enter_context(tc.tile_pool(name="vp", bufs=1))
            oallp = ctx.enter_context(tc.tile_pool(name="oallp", bufs=1))
            wqp = ctx.enter_context(tc.tile_pool(name="wqp", bufs=2))
            wkp = ctx.enter_context(tc.tile_pool(name="wkp", bufs=2))
            qtp = ctx.enter_context(tc.tile_pool(name="qtp", bufs=2))
            ktp = ctx.enter_context(tc.tile_pool(name="ktp", bufs=2))
            estp = ctx.enter_context(tc.tile_pool(name="estp", bufs=6))
            rp = ctx.enter_context(tc.tile_pool(name="rp", bufs=4))
            rbcp = ctx.enter_context(tc.tile_pool(name="rbcp", bufs=2))
            obp = ctx.enter_context(tc.tile_pool(name="obp", bufs=2))
            vgps = ctx.enter_context(tc.tile_pool(name="vgps", bufs=1, space="PSUM"))
            qkps = ctx.enter_context(tc.tile_pool(name="qkps", bufs=2, space="PSUM"))
            stps = ctx.enter_context(tc.tile_pool(name="stps", bufs=2, space="PSUM"))
            ops = ctx.enter_context(tc.tile_pool(name="ops", bufs=2, space="PSUM"))
            # ---- resident constants ----
            wv_sb = const.tile([128, EC * H * VW], FP32R)
            for c in range(EC):
                nc.sync.dma_start(
                    wv_sb[:, c * H * VW:(c + 1) * H * VW],
                    wv.ap()[:, c * H * VW:(c + 1) * H * VW],
                )
            pw_sb = const.tile([128, H * E], FP32R)
            for hc in range(H):
                nc.sync.dma_start(
                    pw_sb[:, hc * E:(hc + 1) * E], pw.ap()[:, hc * E:(hc + 1) * E]
                )
            vb_sb = const.tile([128, H * VW], FP32)
            nc.sync.dma_start(vb_sb[:], vb.ap())
            pb_sb = const.tile([128, E], FP32)
            nc.sync.dma_start(pb_sb[:], pb.ap())
            bqk_sb = const.tile([128, 2 * H], FP32)
            nc.sync.dma_start(bqk_sb[:], bqk.ap())

            for b in range(BPC):
                # ---- x^T for this batch ----
                xt_sb = xtp.tile([128, EC * N], FP32R, tag="xt")
                for c in range(EC):
                    nc.sync.dma_start(
                        xt_sb[:, c * N:(c + 1) * N], xt.ap()[b, :, c * N:(c + 1) * N]
                    )

                # ---- V'' for all heads (bf16): [tok128 x (tokchunk, head, 128)] ----
                v_sb = vp.tile([128, TC * H * VW], BF16, tag="v")
                for t in range(TC):
                    vg = vgps.tile([128, H * VW], FP32, tag="vg")
                    for lo, hi in ((0, 512), (512, H * VW)):
                        for c in range(EC):
                            nc.tensor.matmul(
                                vg[:, lo:hi],
                                xt_sb[:, c * N + t * 128: c * N + (t + 1) * 128],
                                wv_sb[:, c * H * VW + lo: c * H * VW + hi],
                                start=(c == 0),
                                stop=(c == EC - 1),
                            )
                    nc.vector.tensor_tensor(
                        v_sb[:, t * H * VW:(t + 1) * H * VW], vg[:], vb_sb[:],
                        op=OP.add,
                    )

                # ---- O_all^T accumulator in padded-head layout ----
                o_all = oallp.tile([128, H * N], FP32R, tag="oall")

                for h in range(H):
                    wqh = wqp.tile([128, EC * D], FP32R, tag="wq")
                    nc.sync.dma_start(wqh[:], wq.ap()[h])
                    wkh = wkp.tile([128, EC * D], FP32R, tag="wk")
                    nc.sync.dma_start(wkh[:], wk.ap()[h])

                    # Q^T then K^T in 512-col halves (pipelined psum->sbuf copies)
                    qt = qtp.tile([D, N], BF16, tag="qt")
                    kt = ktp.tile([D, N], BF16, tag="kt")
                    for dst, w_sb, bcol in ((qt, wqh, 2 * h), (kt, wkh, 2 * h + 1)):
                        for qh in range(2):
                            g_ps = qkps.tile([D, 512], FP32, tag="qk")
                            for c in range(EC):
                                nc.tensor.matmul(
                                    g_ps[:],
                                    w_sb[:, c * D:(c + 1) * D],
                                    xt_sb[:, c * N + qh * 512: c * N + (qh + 1) * 512],
                                    start=(c == 0),
                                    stop=(c == EC - 1),
                                )
                            nc.scalar.add(
                                dst[:, qh * 512:(qh + 1) * 512], g_ps[:],
                                bqk_sb[0:D, bcol:bcol + 1],
                            )

                    # S^T -> exp -> O, software pipelined over key chunks.
                    # Both q-halves share each weight load (same lhsT).
                    o_ps = [ops.tile([128, 512], FP32, tag="o", name=f"o_{b}_{h}_{i}") for i in range(2)]
                    ests = [[None] * TC, [None] * TC]

                    def s_step(t):
                        for qh in range(2):
                            st = stps.tile([128, 512], FP32, tag="st")
                            nc.tensor.matmul(
                                st[:],
                                kt[:, t * 128:(t + 1) * 128],
                                qt[:, qh * 512:(qh + 1) * 512],
                                start=True,
                                stop=True,
                            )
                            est = estp.tile([128, 512], BF16, tag="est")
                            nc.scalar.activation(est[:], st[:], AF.Exp)
                            ests[qh][t] = est

                    def o_step(t):
                        for qh in range(2):
                            nc.tensor.matmul(
                                o_ps[qh][0:VW, :],
                                v_sb[:, t * H * VW + h * VW: t * H * VW + (h + 1) * VW],
                                ests[qh][t][:],
                                start=(t == 0),
                                stop=(t == TC - 1),
                            )

                    LAT = 1
                    for t in range(TC):
                        s_step(t)
                        if t >= LAT:
                            o_step(t - LAT)
                    for t in range(TC - LAT, TC):
                        o_step(t)

                    for qh in range(2):
                        r = rp.tile([1, 512], FP32, tag="r")
                        rs = rp.tile([1, 512], FP32, tag="rs")
                        nc.vector.reciprocal_approx_accurate(
                            r[:], o_ps[qh][0:1, :], rs[:]
                        )
                        rbc = rbcp.tile([128, 512], FP32, tag="rbc")
                        nc.gpsimd.partition_broadcast(rbc[:], r[:])
                        nc.vector.scalar_tensor_tensor(
                            o_all[0:VW, h * N + qh * 512: h * N + (qh + 1) * 512],
                            o_ps[qh][0:VW, :],
                            SCALE,
                            rbc[0:VW, :],
                            OP.mult,
                            OP.mult,
                        )

                # ---- output projection (psum split across st/o pools) ----
                for t in range(TC):
                    pja = stps.tile([128, 512], FP32, tag="st", name=f"pja_{b}_{t}")
                    pjb = ops.tile([128, 256], FP32, tag="o", name=f"pjb_{b}_{t}")
                    for hc in range(H):
                        lhsT = o_all[0:VW, hc * N + t * 128: hc * N + (t + 1) * 128]
                        nc.tensor.matmul(
                            pja[:], lhsT, pw_sb[0:VW, hc * E: hc * E + 512],
                            start=(hc == 0), stop=(hc == H - 1),
                        )
                        nc.tensor.matmul(
                            pjb[:], lhsT, pw_sb[0:VW, hc * E + 512: hc * E + E],
                            start=(hc == 0), stop=(hc == H - 1),
                        )
                    oba = obp.tile([128, 512], FP32, tag="oba")
                    nc.vector.tensor_tensor(oba[:], pja[:], pb_sb[:, 0:512], op=OP.add)
                    nc.sync.dma_start(out.ap()[b, t * 128:(t + 1) * 128, 0:512], oba[:])
                    obb = obp.tile([128, 256], FP32, tag="obb")
                    nc.vector.tensor_tensor(obb[:], pjb[:], pb_sb[:, 512:E], op=OP.add)
                    nc.sync.dma_start(out.ap()[b, t * 128:(t + 1) * 128, 512:E], obb[:])

    nc.compile()
    return nc


def get_nc():
    if "nc" not in _NC_CACHE:
        _NC_CACHE["nc"] = _build_nc()
    return _NC_CACHE["nc"]


def _prep_inputs(x, qkv_w, qkv_b, proj_w, proj_b):
    """Host-side layout prep shared by all cores + per-core x shards."""
    x = np.ascontiguousarray(x, dtype=np.float32)
    qkv_w = np.asarray(qkv_w, dtype=np.float32)
    qkv_b = np.asarray(qkv_b, dtype=np.float32)
    proj_w = np.asarray(proj_w, dtype=np.float32)
    proj_b = np.asarray(proj_b, dtype=np.float32)

    hh = np.arange(H)[:, None]
    dd = np.arange(D)[None, :]
    idx = [(hh * 3 * D + dd * 3 + c).reshape(-1) for c in range(3)]  # [768] each

    # wq/wk: [H, 128, EC*D]; wX_l[h, p, c*D+d] = qkv_w[idx[c_][h*D+d], c*128+p]
    def qk_layout(cix):
        wT = qkv_w[idx[cix], :].T  # [E, 768] : [e, h*D+d]
        w = wT.reshape(EC, 128, H, D)          # [c, p, h, d]
        return np.ascontiguousarray(w.transpose(2, 1, 0, 3).reshape(H, 128, EC * D))

    wq_l = qk_layout(0)
    wk_l = qk_layout(1)

    # wv: [128, EC*H*DP]; col c*H*DP + h*DP + d = qkv_w[idx2[h*D+d], c*128+p], pad 0
    wvT = qkv_w[idx[2], :].T.reshape(EC, 128, H, D)  # [c, p, h, d]
    wv_l = np.zeros((128, EC, H, VW), dtype=np.float32)
    wv_l[:, :, :, 1:D + 1] = wvT.transpose(1, 0, 2, 3)
    wv_l = np.ascontiguousarray(wv_l.reshape(128, EC * H * VW))

    # vb: [128, H*DP] broadcast v-bias + ones column at d=D
    vb_row = np.zeros((H, VW), dtype=np.float32)
    vb_row[:, 1:D + 1] = qkv_b[idx[2]].reshape(H, D)
    vb_row[:, 0] = 1.0
    vb_l = np.ascontiguousarray(np.broadcast_to(vb_row.reshape(1, H * VW), (128, H * VW)))

    # bqk: [128, 2H]; col 2h = q bias, col 2h+1 = k bias (rows 0..D-1)
    bqk_l = np.zeros((128, 2 * H), dtype=np.float32)
    bqk_l[:D, 0::2] = qkv_b[idx[0]].reshape(H, D).T
    bqk_l[:D, 1::2] = qkv_b[idx[1]].reshape(H, D).T

    # pw: [128, H*E]; pw_l[p, h*E+e] = proj_w[e, h*D+dd] for p=dd<D else 0
    pw_l = np.zeros((128, H, E), dtype=np.float32)
    pw_l[1:D + 1, :, :] = proj_w.reshape(E, H, D).transpose(2, 1, 0)
    pw_l = np.ascontiguousarray(pw_l.reshape(128, H * E))

    pb_l = np.ascontiguousarray(np.broadcast_to(proj_b.reshape(1, E), (128, E)))

    # x^T per batch in sbuf layout: [B, 128, EC*N]; [b, p, c*N+n] = x[b, n, c*128+p]
    xt_all = np.ascontiguousarray(
        x.reshape(B, N, EC, 128).transpose(0, 3, 2, 1).reshape(B, 128, EC * N)
    )

    in_maps = []
    for core in range(NCORES):
        xt_core = np.ascontiguousarray(
            xt_all[core * BPC:(core + 1) * BPC]
        )
        in_maps.append(
            {
                "xt": xt_core,
                "wq": wq_l,
                "wk": wk_l,
                "wv": wv_l,
                "vb": vb_l,
                "bqk": bqk_l,
                "pw": pw_l,
                "pb": pb_l,
            }
        )
    return in_maps


def run(inputs, trace=False):
    from concourse.bass_utils import run_bass_kernel_spmd

    nc = get_nc()
    in_maps = _prep_inputs(**inputs)
    res = run_bass_kernel_spmd(
        nc, in_maps, core_ids=list(range(NCORES)), trace=trace
    )
    out = np.concatenate([res.results[c]["out"] for c in range(NCORES)], axis=0)
    return out, res


def kernel(**inputs) -> np.ndarray:
    out, _ = run(inputs, trace=False)
    return out
